# revision 25
# baseline (speedup 1.0000x reference)
"""Bass/Trainium2 kernel for nn_BoxFilter: 9x9 circular box-mean over
(8, 3, 1024, 1024) f32, data-parallel across 8 NeuronCores (1 image/core).

All-bf16 I/O pipeline (rel err ~6e-3, gate is 2e-2), ~12.5 MB DMA/core:
  - host packs each image circularly padded: [3, 1036, 1032] bf16
    (rows/cols pre-wrapped, so no wraparound DMAs on device)
  - PE: v3 = vertical-9 x horizontal-3 sums via 3 column-shifted
    accumulating matmuls per PSUM chunk; one stationary band-weight
    matrix serves every block (k=128 overlap windows)
  - ACT: PSUM -> SBUF drain with x(1/81) scale, bf16 out
  - DVE: out[n] = u3[n] + u3[n+3] + u3[n+6] in two bf16 tensor_tensor
    passes (2x_1p mode), block-paired to halve instruction overhead
  - loads on the SP HWDGE ring (channel 0 split so compute starts
    early); stores issued per block-pair (480 KB), alternating between
    the ACT HWDGE ring and GPSIMD SWDGE so the tail drains fast
"""

import numpy as np
import ml_dtypes

import concourse.bacc as bacc
import concourse.mybir as mybir
import concourse.tile as tile
from concourse.ap import AP
from concourse.bass_utils import run_bass_kernel_spmd

B, C, H, W = 8, 3, 1024, 1024
R = 4             # filter radius
WIN = 2 * R + 1   # 9
AREA = WIN * WIN  # 81
M = 120           # output rows per full block (input window = 128 rows)
NB = 8            # full blocks per channel
MT = H - NB * M   # 64: tail block output rows
KT = MT + 2 * R   # 72: tail block input rows
RPAD = H + R + 8  # 1036 padded rows: padded row i == real row (i-4) mod H
CP = W + 2 * R    # 1032 padded cols: padded col j == real col (j-4) mod W
U3 = CP - 2       # 1030 u3 columns per block
# PSUM chunking: each matmul output must stay within one 512-f32 PSUM bank.
# Main tile vA holds cols [0:1024) (exactly 2 banks, 3 buffers for a deep
# PE runway); the 6-col remainder for 4 blocks shares one 1-bank tile vB.
CHUNKS = ((0, 512), (512, 512))
CREM = U3 - 1024  # 6

_CACHE: dict = {}


def _band_weights() -> np.ndarray:
    w = np.zeros((128, M), dtype=ml_dtypes.bfloat16)
    for m in range(M):
        w[m : m + WIN, m] = 1.0
    return w


def _pack_image(x: np.ndarray) -> np.ndarray:
    """[C,H,W] f32 -> [C, 1036, 1032] bf16, circularly padded by R=4
    (rows: 4 top / 8 bottom, cols: 4 each side)."""
    rows = (np.arange(RPAD) - R) % H
    cols = (np.arange(CP) - R) % W
    xp = x[:, rows][:, :, cols]
    return np.ascontiguousarray(xp.astype(ml_dtypes.bfloat16))


def _build():
    f32 = mybir.dt.float32
    bf16 = mybir.dt.bfloat16
    add = mybir.AluOpType.add
    nc = bacc.Bacc("TRN2", target_bir_lowering=False, debug=False, num_devices=B)
    x_d = nc.dram_tensor("x", [C, RPAD, CP], bf16, kind="ExternalInput")
    w_d = nc.dram_tensor("w", [128, M], bf16, kind="ExternalInput")
    o_d = nc.dram_tensor("o", [C, H, W], bf16, kind="ExternalOutput")
    XCH = RPAD * CP  # elements per packed channel
    OCH = H * W      # elements per output channel

    with tile.TileContext(nc) as tc:
        with (
            tc.tile_pool(name="wpool", bufs=1) as wpool,
            tc.tile_pool(name="xpool", bufs=3) as xpool,
            tc.tile_pool(name="upool", bufs=4) as upool,
            tc.tile_pool(name="tpool", bufs=3) as tpool,
            tc.tile_pool(name="opool", bufs=2) as opool,
            tc.tile_pool(name="psum", bufs=3, space="PSUM") as psum,
            tc.tile_pool(name="psumb", bufs=2, space="PSUM") as psumb,
        ):
            QB = 4  # blocks per u3/combine group
            w_t = wpool.tile([128, M], bf16)
            # weight load on the ACT ring, in parallel with window-0 on sync
            nc.scalar.dma_start(w_t[:], w_d.ap())

            def load_channel(c, split_first):
                x_t = xpool.tile([128, NB + 1, CP], bf16, tag="x")
                # windows 0..7: partition p, window w <- padded row 120w + p
                if split_first:
                    # progressive: window 0 lands first, then 1-3, then 4-7
                    nc.sync.dma_start(
                        x_t[0:128, 0:1, :], AP(x_d, c * XCH, [[CP, 128], [1, CP]])
                    )
                    nc.sync.dma_start(
                        x_t[0:128, 1:4, :],
                        AP(x_d, c * XCH + M * CP, [[CP, 128], [M * CP, 3], [1, CP]]),
                    )
                    nc.sync.dma_start(
                        x_t[0:128, 4:NB, :],
                        AP(x_d, c * XCH + 4 * M * CP, [[CP, 128], [M * CP, NB - 4], [1, CP]]),
                    )
                else:
                    nc.sync.dma_start(
                        x_t[0:128, 0:NB, :],
                        AP(x_d, c * XCH, [[CP, 128], [M * CP, NB], [1, CP]]),
                    )
                # tail window: padded rows 960..1031 (72 rows)
                nc.sync.dma_start(
                    x_t[0:KT, NB, :],
                    AP(x_d, c * XCH + NB * M * CP, [[CP, KT], [1, CP]]),
                )
                return x_t

            def vertical3(x_t, j, m, k):
                """6 matmuls: v3[mm, i] = sum_{d=0..2} sum_kk band(kk,mm) x[kk, j, i+d]
                for cols [0:1024) into a fresh 2-bank PSUM tile."""
                vA = psum.tile([128, 1024], f32, tag="vA")
                for c0, cn in CHUNKS:
                    for d in range(3):
                        nc.tensor.matmul(
                            vA[0:m, c0 : c0 + cn],
                            w_t[0:k, 0:m],
                            x_t[0:k, j, c0 + d : c0 + d + cn],
                            start=(d == 0),
                            stop=(d == 2),
                        )
                return vA

            def tiny3(x_t, j0, nj, m, k, vb_t):
                """Cols [1024:1030) of nj consecutive blocks in 3 matmuls:
                the band weights are window-independent, so one matmul spans
                all nj windows via a 3D moving AP."""
                for d in range(3):
                    nc.tensor.matmul(
                        vb_t[0:m, 0:nj, 0:CREM],
                        w_t[0:k, 0:m],
                        x_t[0:k, j0 : j0 + nj, 1024 + d : 1024 + d + CREM],
                        start=(d == 0),
                        stop=(d == 2),
                    )

            def drain_big(vA, u3_t, q, m):
                nc.scalar.mul(
                    out=u3_t[0:m, q, 0:1024], in_=vA[0:m, 0:1024], mul=1.0 / AREA
                )

            def drain_small(vb_t, u3_t, nq, m):
                nc.scalar.mul(
                    out=u3_t[0:m, 0:nq, 1024:U3],
                    in_=vb_t[0:m, 0:nq, 0:CREM],
                    mul=1.0 / AREA,
                )

            def combine(u3_t, o_t, j0, nq, m, q0=0, gp_last=False):
                """out[n] = u3[n] + u3[n+3] + u3[n+6] over nq stacked blocks.
                gp_last: the last block's pass-2 runs on GPSIMD."""
                t_t = tpool.tile([128, QB, CP], bf16, tag="t")
                nc.vector.tensor_tensor(
                    out=t_t[0:m, 0:nq, 0:W],
                    in0=u3_t[0:m, q0 : q0 + nq, 0:W],
                    in1=u3_t[0:m, q0 : q0 + nq, 3 : W + 3],
                    op=add,
                )
                h = nq - 1 if (gp_last and nq > 1) else nq
                nc.vector.tensor_tensor(
                    out=o_t[0:m, j0 : j0 + h, :],
                    in0=t_t[0:m, 0:h, 0:W],
                    in1=u3_t[0:m, q0 : q0 + h, 6:U3],
                    op=add,
                )
                if h < nq:
                    nc.gpsimd.tensor_tensor(
                        out=o_t[0:m, j0 + h : j0 + nq, :],
                        in0=t_t[0:m, h:nq, 0:W],
                        in1=u3_t[0:m, q0 + h : q0 + nq, 6:U3],
                        op=add,
                    )

            def store_rows(c, o_t, j0, nj, eng):
                eng.dma_start(
                    AP(o_d, c * OCH + j0 * M * W, [[W, M], [M * W, nj], [1, W]]),
                    o_t[0:M, j0 : j0 + nj, :],
                )

            def do_tail(c, x_t, o_t, store_eng):
                u3_t = upool.tile([128, QB, CP], bf16, tag="u3")
                vb_t = psumb.tile([128, QB, CREM], f32, tag="vB")
                tiny3(x_t, NB, 1, MT, KT, vb_t)
                drain_small(vb_t, u3_t, 1, MT)
                vA = vertical3(x_t, NB, MT, KT)
                drain_big(vA, u3_t, 0, MT)
                combine(u3_t, o_t, NB, 1, MT)
                store_eng.dma_start(
                    AP(o_d, c * OCH + NB * M * W, [[W, MT], [1, W]]),
                    o_t[0:MT, NB, :],
                )

            def do_quad(c, x_t, o_t, g, store_eng):
                u3_t = upool.tile([128, QB, CP], bf16, tag="u3")
                vb_t = psumb.tile([128, QB, CREM], f32, tag="vB")
                tiny3(x_t, QB * g, QB, M, 128, vb_t)
                drain_small(vb_t, u3_t, QB, M)
                for q in range(QB):
                    vA = vertical3(x_t, QB * g + q, M, 128)
                    drain_big(vA, u3_t, q, M)
                combine(u3_t, o_t, QB * g, QB, M, gp_last=True)
                store_rows(c, o_t, QB * g, QB, store_eng)

            x_tiles = [load_channel(c, split_first=(c == 0)) for c in range(C)]
            for c in range(C - 1):
                x_t = x_tiles[c]
                o_t = opool.tile([128, NB + 1, W], bf16, tag="o")
                do_quad(c, x_t, o_t, 0, nc.sync)
                do_quad(c, x_t, o_t, 1, nc.gpsimd)
                do_tail(c, x_t, o_t, nc.gpsimd)
            # last channel: shrinking groups, 64-row tail last -> short pipe tail
            c = C - 1
            x_t = x_tiles[c]
            o_t = opool.tile([128, NB + 1, W], bf16, tag="o")
            do_quad(c, x_t, o_t, 0, nc.gpsimd)
            u3_t = upool.tile([128, QB, CP], bf16, tag="u3")
            vb_t = psumb.tile([128, QB, CREM], f32, tag="vB")
            tiny3(x_t, 4, 2, M, 128, vb_t)
            drain_small(vb_t, u3_t, 2, M)
            for q in range(2):
                vA = vertical3(x_t, 4 + q, M, 128)
                drain_big(vA, u3_t, q, M)
            combine(u3_t, o_t, 4, 2, M)
            store_rows(c, o_t, 4, 2, nc.gpsimd)
            u3_t = upool.tile([128, QB, CP], bf16, tag="u3")
            vb_t = psumb.tile([128, QB, CREM], f32, tag="vB")
            tiny3(x_t, 6, 2, M, 128, vb_t)
            drain_small(vb_t, u3_t, 2, M)
            for q in range(2):
                vA = vertical3(x_t, 6 + q, M, 128)
                drain_big(vA, u3_t, q, M)
                combine(u3_t, o_t, 6 + q, 1, M, q0=q)
                store_rows(c, o_t, 6 + q, 1, nc.scalar if q else nc.sync)
            do_tail(c, x_t, o_t, nc.sync)
    nc.compile()
    return nc


def _get_nc():
    if "nc" not in _CACHE:
        _CACHE["nc"] = _build()
    return _CACHE["nc"]


def _prepare_in_maps(tensor: np.ndarray) -> list:
    x = np.asarray(tensor, dtype=np.float32)
    assert x.shape == (B, C, H, W), x.shape
    wmat = _band_weights()
    return [{"x": _pack_image(x[i]), "w": wmat} for i in range(B)]


def kernel(tensor: np.ndarray) -> np.ndarray:
    nc = _get_nc()
    in_maps = _prepare_in_maps(tensor)
    res = run_bass_kernel_spmd(nc, in_maps, core_ids=list(range(B)))
    return np.stack(
        [res.results[i]["o"].astype(np.float32) for i in range(B)], axis=0
    )


# revision 32
# speedup vs baseline: 1.0814x; 1.0814x over previous
"""Bass/Trainium2 kernel for nn_BoxFilter: 9x9 circular box-mean over
(8, 3, 1024, 1024) f32, data-parallel across 8 NeuronCores (1 image/core).

All-bf16 I/O pipeline (rel err ~6e-3, gate is 2e-2), ~12.5 MB DMA/core:
  - host packs each image circularly padded: [3, 1036, 1032] bf16
    (rows/cols pre-wrapped, so no wraparound DMAs on device)
  - PE: v3 = vertical-9 x horizontal-3 sums via 3 column-shifted
    accumulating matmuls per PSUM chunk; one stationary band-weight
    matrix serves every block (k=128 overlap windows)
  - ACT: PSUM -> SBUF drain with x(1/81) scale, bf16 out
  - DVE: out[n] = u3[n] + u3[n+3] + u3[n+6] in two bf16 tensor_tensor
    passes (2x_1p mode), block-paired to halve instruction overhead
  - loads on the SP HWDGE ring (channel 0 split so compute starts
    early); stores issued per block-pair (480 KB), alternating between
    the ACT HWDGE ring and GPSIMD SWDGE so the tail drains fast
"""

import numpy as np
import ml_dtypes

import concourse.bacc as bacc
import concourse.mybir as mybir
import concourse.tile as tile
from concourse.ap import AP
from concourse.bass_utils import run_bass_kernel_spmd

B, C, H, W = 8, 3, 1024, 1024
R = 4             # filter radius
WIN = 2 * R + 1   # 9
AREA = WIN * WIN  # 81
M = 120           # output rows per full block (input window = 128 rows)
NB = 8            # full blocks per channel
MT = H - NB * M   # 64: tail block output rows
KT = MT + 2 * R   # 72: tail block input rows
RPAD = H + R + 8  # 1036 padded rows: padded row i == real row (i-4) mod H
CP = W + 2 * R    # 1032 padded cols: padded col j == real col (j-4) mod W
U3 = CP - 2       # 1030 u3 columns per block
# PSUM chunking: each matmul output must stay within one 512-f32 PSUM bank.
# Main tile vA holds cols [0:1024) (exactly 2 banks, 3 buffers for a deep
# PE runway); the 6-col remainder for 4 blocks shares one 1-bank tile vB.
CHUNKS = ((0, 512), (512, 512))
CREM = U3 - 1024  # 6

_CACHE: dict = {}


def _band_weights() -> np.ndarray:
    w = np.zeros((128, M), dtype=ml_dtypes.bfloat16)
    for m in range(M):
        w[m : m + WIN, m] = 1.0
    return w


def _pack_image(x: np.ndarray) -> np.ndarray:
    """[C,H,W] f32 -> [C, 1036, 1032] bf16, circularly padded by R=4
    (rows: 4 top / 8 bottom, cols: 4 each side)."""
    rows = (np.arange(RPAD) - R) % H
    cols = (np.arange(CP) - R) % W
    xp = x[:, rows][:, :, cols]
    return np.ascontiguousarray(xp.astype(ml_dtypes.bfloat16))


def _build():
    f32 = mybir.dt.float32
    bf16 = mybir.dt.bfloat16
    add = mybir.AluOpType.add
    nc = bacc.Bacc("TRN2", target_bir_lowering=False, debug=False, num_devices=B)
    x_d = nc.dram_tensor("x", [C, RPAD, CP], bf16, kind="ExternalInput")
    w_d = nc.dram_tensor("w", [128, M], bf16, kind="ExternalInput")
    o_d = nc.dram_tensor("o", [C, H, W], bf16, kind="ExternalOutput")
    XCH = RPAD * CP  # elements per packed channel
    OCH = H * W      # elements per output channel

    with tile.TileContext(nc) as tc:
        with (
            tc.tile_pool(name="wpool", bufs=1) as wpool,
            tc.tile_pool(name="xpool", bufs=3) as xpool,
            tc.tile_pool(name="upool", bufs=4) as upool,
            tc.tile_pool(name="tpool", bufs=3) as tpool,
            tc.tile_pool(name="opool", bufs=2) as opool,
            tc.tile_pool(name="psum", bufs=3, space="PSUM") as psum,
            tc.tile_pool(name="psumb", bufs=2, space="PSUM") as psumb,
        ):
            QB = 4  # blocks per u3/combine group
            w_t = wpool.tile([128, M], bf16)
            nc.sync.dma_start(w_t[:], w_d.ap())

            def load_channel(c, split_first):
                x_t = xpool.tile([128, NB + 1, CP], bf16, tag="x")
                # windows 0..7: partition p, window w <- padded row 120w + p
                if split_first:
                    # progressive: window 0 lands first, then 1-3, then 4-7
                    nc.sync.dma_start(
                        x_t[0:128, 0:1, :], AP(x_d, c * XCH, [[CP, 128], [1, CP]])
                    )
                    nc.sync.dma_start(
                        x_t[0:128, 1:4, :],
                        AP(x_d, c * XCH + M * CP, [[CP, 128], [M * CP, 3], [1, CP]]),
                    )
                    nc.sync.dma_start(
                        x_t[0:128, 4:NB, :],
                        AP(x_d, c * XCH + 4 * M * CP, [[CP, 128], [M * CP, NB - 4], [1, CP]]),
                    )
                else:
                    nc.sync.dma_start(
                        x_t[0:128, 0:NB, :],
                        AP(x_d, c * XCH, [[CP, 128], [M * CP, NB], [1, CP]]),
                    )
                # tail window: padded rows 960..1031 (72 rows)
                nc.sync.dma_start(
                    x_t[0:KT, NB, :],
                    AP(x_d, c * XCH + NB * M * CP, [[CP, KT], [1, CP]]),
                )
                return x_t

            def vertical3(x_t, j, m, k):
                """6 matmuls: v3[mm, i] = sum_{d=0..2} sum_kk band(kk,mm) x[kk, j, i+d]
                for cols [0:1024) into a fresh 2-bank PSUM tile."""
                vA = psum.tile([128, 1024], f32, tag="vA")
                for c0, cn in CHUNKS:
                    for d in range(3):
                        nc.tensor.matmul(
                            vA[0:m, c0 : c0 + cn],
                            w_t[0:k, 0:m],
                            x_t[0:k, j, c0 + d : c0 + d + cn],
                            start=(d == 0),
                            stop=(d == 2),
                        )
                return vA

            def tiny3(x_t, j0, nj, m, k, vb_t):
                """Cols [1024:1030) of nj consecutive blocks in 3 matmuls:
                the band weights are window-independent, so one matmul spans
                all nj windows via a 3D moving AP."""
                for d in range(3):
                    nc.tensor.matmul(
                        vb_t[0:m, 0:nj, 0:CREM],
                        w_t[0:k, 0:m],
                        x_t[0:k, j0 : j0 + nj, 1024 + d : 1024 + d + CREM],
                        start=(d == 0),
                        stop=(d == 2),
                    )

            def drain_big(vA, u3_t, q, m):
                nc.scalar.mul(
                    out=u3_t[0:m, q, 0:1024], in_=vA[0:m, 0:1024], mul=1.0 / AREA
                )

            def drain_small(vb_t, u3_t, nq, m):
                nc.scalar.mul(
                    out=u3_t[0:m, 0:nq, 1024:U3],
                    in_=vb_t[0:m, 0:nq, 0:CREM],
                    mul=1.0 / AREA,
                )

            def combine(u3_t, o_t, j0, nq, m, q0=0, gp_last=False):
                """out[n] = u3[n] + u3[n+3] + u3[n+6] over nq stacked blocks.
                gp_last: the last block's pass-2 runs on GPSIMD."""
                t_t = tpool.tile([128, QB, CP], bf16, tag="t")
                nc.vector.tensor_tensor(
                    out=t_t[0:m, 0:nq, 0:W],
                    in0=u3_t[0:m, q0 : q0 + nq, 0:W],
                    in1=u3_t[0:m, q0 : q0 + nq, 3 : W + 3],
                    op=add,
                )
                h = nq - 1 if (gp_last and nq > 1) else nq
                nc.vector.tensor_tensor(
                    out=o_t[0:m, j0 : j0 + h, :],
                    in0=t_t[0:m, 0:h, 0:W],
                    in1=u3_t[0:m, q0 : q0 + h, 6:U3],
                    op=add,
                )
                if h < nq:
                    nc.gpsimd.tensor_tensor(
                        out=o_t[0:m, j0 + h : j0 + nq, :],
                        in0=t_t[0:m, h:nq, 0:W],
                        in1=u3_t[0:m, q0 + h : q0 + nq, 6:U3],
                        op=add,
                    )

            def store_rows(c, o_t, j0, nj, eng):
                eng.dma_start(
                    AP(o_d, c * OCH + j0 * M * W, [[W, M], [M * W, nj], [1, W]]),
                    o_t[0:M, j0 : j0 + nj, :],
                )

            def do_tail(c, x_t, o_t, store_eng):
                u3_t = upool.tile([128, QB, CP], bf16, tag="u3")
                vb_t = psumb.tile([128, QB, CREM], f32, tag="vB")
                tiny3(x_t, NB, 1, MT, KT, vb_t)
                drain_small(vb_t, u3_t, 1, MT)
                vA = vertical3(x_t, NB, MT, KT)
                drain_big(vA, u3_t, 0, MT)
                combine(u3_t, o_t, NB, 1, MT)
                store_eng.dma_start(
                    AP(o_d, c * OCH + NB * M * W, [[W, MT], [1, W]]),
                    o_t[0:MT, NB, :],
                )

            def do_quad(c, x_t, o_t, g, store_eng, gp_last=True):
                u3_t = upool.tile([128, QB, CP], bf16, tag="u3")
                vb_t = psumb.tile([128, QB, CREM], f32, tag="vB")
                tiny3(x_t, QB * g, QB, M, 128, vb_t)
                drain_small(vb_t, u3_t, QB, M)
                for q in range(QB):
                    vA = vertical3(x_t, QB * g + q, M, 128)
                    drain_big(vA, u3_t, q, M)
                combine(u3_t, o_t, QB * g, QB, M, gp_last=gp_last)
                store_rows(c, o_t, QB * g, QB, store_eng)

            x_tiles = [load_channel(c, split_first=(c == 0)) for c in range(C)]
            for c in range(C - 1):
                x_t = x_tiles[c]
                o_t = opool.tile([128, NB + 1, W], bf16, tag="o")
                do_quad(c, x_t, o_t, 0, nc.sync)
                do_quad(c, x_t, o_t, 1, nc.gpsimd)
                do_tail(c, x_t, o_t, nc.gpsimd)
            # last channel: tail first, then shrinking groups -> short pipe tail
            c = C - 1
            x_t = x_tiles[c]
            o_t = opool.tile([128, NB + 1, W], bf16, tag="o")
            do_tail(c, x_t, o_t, nc.gpsimd)
            do_quad(c, x_t, o_t, 0, nc.gpsimd)
            u3_t = upool.tile([128, QB, CP], bf16, tag="u3")
            vb_t = psumb.tile([128, QB, CREM], f32, tag="vB")
            tiny3(x_t, 4, 2, M, 128, vb_t)
            drain_small(vb_t, u3_t, 2, M)
            for q in range(2):
                vA = vertical3(x_t, 4 + q, M, 128)
                drain_big(vA, u3_t, q, M)
            combine(u3_t, o_t, 4, 2, M)
            store_rows(c, o_t, 4, 2, nc.sync)
            u3_t = upool.tile([128, QB, CP], bf16, tag="u3")
            vb_t = psumb.tile([128, QB, CREM], f32, tag="vB")
            tiny3(x_t, 6, 2, M, 128, vb_t)
            drain_small(vb_t, u3_t, 2, M)
            for q in range(2):
                vA = vertical3(x_t, 6 + q, M, 128)
                drain_big(vA, u3_t, q, M)
                combine(u3_t, o_t, 6 + q, 1, M, q0=q)
                store_rows(c, o_t, 6 + q, 1, nc.scalar if q else nc.sync)
    nc.compile()
    return nc


def _get_nc():
    if "nc" not in _CACHE:
        _CACHE["nc"] = _build()
    return _CACHE["nc"]


def _prepare_in_maps(tensor: np.ndarray) -> list:
    x = np.asarray(tensor, dtype=np.float32)
    assert x.shape == (B, C, H, W), x.shape
    wmat = _band_weights()
    return [{"x": _pack_image(x[i]), "w": wmat} for i in range(B)]


def kernel(tensor: np.ndarray) -> np.ndarray:
    nc = _get_nc()
    in_maps = _prepare_in_maps(tensor)
    res = run_bass_kernel_spmd(nc, in_maps, core_ids=list(range(B)))
    return np.stack(
        [res.results[i]["o"].astype(np.float32) for i in range(B)], axis=0
    )


# revision 34
# speedup vs baseline: 1.1154x; 1.0314x over previous
"""Bass/Trainium2 kernel for nn_BoxFilter: 9x9 circular box-mean over
(8, 3, 1024, 1024) f32, data-parallel across 8 NeuronCores (1 image/core).

All-bf16 I/O pipeline (rel err ~6e-3, gate is 2e-2), ~12.5 MB DMA/core:
  - host packs each image circularly padded: [3, 1036, 1032] bf16
    (rows/cols pre-wrapped, so no wraparound DMAs on device)
  - PE: v3 = vertical-9 x horizontal-3 sums via 3 column-shifted
    accumulating matmuls per PSUM chunk; one stationary band-weight
    matrix serves every block (k=128 overlap windows)
  - ACT: PSUM -> SBUF drain with x(1/81) scale, bf16 out
  - DVE: out[n] = u3[n] + u3[n+3] + u3[n+6] in two bf16 tensor_tensor
    passes (2x_1p mode), block-paired to halve instruction overhead
  - loads on the SP HWDGE ring (channel 0 split so compute starts
    early); stores issued per block-pair (480 KB), alternating between
    the ACT HWDGE ring and GPSIMD SWDGE so the tail drains fast
"""

import numpy as np
import ml_dtypes

import concourse.bacc as bacc
import concourse.mybir as mybir
import concourse.tile as tile
from concourse.ap import AP
from concourse.bass_utils import run_bass_kernel_spmd

B, C, H, W = 8, 3, 1024, 1024
R = 4             # filter radius
WIN = 2 * R + 1   # 9
AREA = WIN * WIN  # 81
M = 120           # output rows per full block (input window = 128 rows)
NB = 8            # full blocks per channel
MT = H - NB * M   # 64: tail block output rows
KT = MT + 2 * R   # 72: tail block input rows
RPAD = H + R + 8  # 1036 padded rows: padded row i == real row (i-4) mod H
CP = W + 2 * R    # 1032 padded cols: padded col j == real col (j-4) mod W
U3 = CP - 2       # 1030 u3 columns per block
# PSUM chunking: each matmul output must stay within one 512-f32 PSUM bank.
# Main tile vA holds cols [0:1024) (exactly 2 banks, 3 buffers for a deep
# PE runway); the 6-col remainder for 4 blocks shares one 1-bank tile vB.
CHUNKS = ((0, 512), (512, 512))
CREM = U3 - 1024  # 6

_CACHE: dict = {}


def _band_weights() -> np.ndarray:
    w = np.zeros((128, M), dtype=ml_dtypes.bfloat16)
    for m in range(M):
        w[m : m + WIN, m] = 1.0
    return w


def _pack_image(x: np.ndarray) -> np.ndarray:
    """[C,H,W] f32 -> [C, 1036, 1032] bf16, circularly padded by R=4
    (rows: 4 top / 8 bottom, cols: 4 each side)."""
    rows = (np.arange(RPAD) - R) % H
    cols = (np.arange(CP) - R) % W
    xp = x[:, rows][:, :, cols]
    return np.ascontiguousarray(xp.astype(ml_dtypes.bfloat16))


def _build():
    f32 = mybir.dt.float32
    bf16 = mybir.dt.bfloat16
    add = mybir.AluOpType.add
    nc = bacc.Bacc("TRN2", target_bir_lowering=False, debug=False, num_devices=B)
    x_d = nc.dram_tensor("x", [C, RPAD, CP], bf16, kind="ExternalInput")
    w_d = nc.dram_tensor("w", [128, M], bf16, kind="ExternalInput")
    o_d = nc.dram_tensor("o", [C, H, W], bf16, kind="ExternalOutput")
    XCH = RPAD * CP  # elements per packed channel
    OCH = H * W      # elements per output channel

    with tile.TileContext(nc) as tc:
        with (
            tc.tile_pool(name="wpool", bufs=1) as wpool,
            tc.tile_pool(name="xpool", bufs=3) as xpool,
            tc.tile_pool(name="upool", bufs=4) as upool,
            tc.tile_pool(name="tpool", bufs=3) as tpool,
            tc.tile_pool(name="opool", bufs=2) as opool,
            tc.tile_pool(name="psum", bufs=3, space="PSUM") as psum,
            tc.tile_pool(name="psumb", bufs=2, space="PSUM") as psumb,
        ):
            QB = 4  # blocks per u3/combine group
            w_t = wpool.tile([128, M], bf16)
            nc.sync.dma_start(w_t[:], w_d.ap())

            def load_channel(c, split_first):
                x_t = xpool.tile([128, NB + 1, CP], bf16, tag="x")
                # windows 0..7: partition p, window w <- padded row 120w + p
                if split_first:
                    # progressive: window 0 lands first, then 1-3, then 4-7
                    nc.sync.dma_start(
                        x_t[0:128, 0:1, :], AP(x_d, c * XCH, [[CP, 128], [1, CP]])
                    )
                    nc.sync.dma_start(
                        x_t[0:128, 1:4, :],
                        AP(x_d, c * XCH + M * CP, [[CP, 128], [M * CP, 3], [1, CP]]),
                    )
                    nc.sync.dma_start(
                        x_t[0:128, 4:NB, :],
                        AP(x_d, c * XCH + 4 * M * CP, [[CP, 128], [M * CP, NB - 4], [1, CP]]),
                    )
                else:
                    nc.sync.dma_start(
                        x_t[0:128, 0:NB, :],
                        AP(x_d, c * XCH, [[CP, 128], [M * CP, NB], [1, CP]]),
                    )
                # tail window: padded rows 960..1031 (72 rows)
                nc.sync.dma_start(
                    x_t[0:KT, NB, :],
                    AP(x_d, c * XCH + NB * M * CP, [[CP, KT], [1, CP]]),
                )
                return x_t

            def vertical3(x_t, j, m, k):
                """6 matmuls: v3[mm, i] = sum_{d=0..2} sum_kk band(kk,mm) x[kk, j, i+d]
                for cols [0:1024) into a fresh 2-bank PSUM tile."""
                vA = psum.tile([128, 1024], f32, tag="vA")
                for c0, cn in CHUNKS:
                    for d in range(3):
                        nc.tensor.matmul(
                            vA[0:m, c0 : c0 + cn],
                            w_t[0:k, 0:m],
                            x_t[0:k, j, c0 + d : c0 + d + cn],
                            start=(d == 0),
                            stop=(d == 2),
                        )
                return vA

            def tiny3(x_t, j0, nj, m, k, vb_t):
                """Cols [1024:1030) of nj consecutive blocks in 3 matmuls:
                the band weights are window-independent, so one matmul spans
                all nj windows via a 3D moving AP."""
                for d in range(3):
                    nc.tensor.matmul(
                        vb_t[0:m, 0:nj, 0:CREM],
                        w_t[0:k, 0:m],
                        x_t[0:k, j0 : j0 + nj, 1024 + d : 1024 + d + CREM],
                        start=(d == 0),
                        stop=(d == 2),
                    )

            def drain_big(vA, u3_t, q, m):
                nc.scalar.mul(
                    out=u3_t[0:m, q, 0:1024], in_=vA[0:m, 0:1024], mul=1.0 / AREA
                )

            def drain_small(vb_t, u3_t, nq, m, vq=0):
                nc.scalar.mul(
                    out=u3_t[0:m, 0:nq, 1024:U3],
                    in_=vb_t[0:m, vq : vq + nq, 0:CREM],
                    mul=1.0 / AREA,
                )

            def combine(u3_t, o_t, j0, nq, m, q0=0, gp_last=False):
                """out[n] = u3[n] + u3[n+3] + u3[n+6] over nq stacked blocks.
                gp_last: the last block's pass-2 runs on GPSIMD."""
                t_t = tpool.tile([128, QB, CP], bf16, tag="t")
                nc.vector.tensor_tensor(
                    out=t_t[0:m, 0:nq, 0:W],
                    in0=u3_t[0:m, q0 : q0 + nq, 0:W],
                    in1=u3_t[0:m, q0 : q0 + nq, 3 : W + 3],
                    op=add,
                )
                h = nq - 1 if (gp_last and nq > 1) else nq
                nc.vector.tensor_tensor(
                    out=o_t[0:m, j0 : j0 + h, :],
                    in0=t_t[0:m, 0:h, 0:W],
                    in1=u3_t[0:m, q0 : q0 + h, 6:U3],
                    op=add,
                )
                if h < nq:
                    nc.gpsimd.tensor_tensor(
                        out=o_t[0:m, j0 + h : j0 + nq, :],
                        in0=t_t[0:m, h:nq, 0:W],
                        in1=u3_t[0:m, q0 + h : q0 + nq, 6:U3],
                        op=add,
                    )

            def store_rows(c, o_t, j0, nj, eng):
                eng.dma_start(
                    AP(o_d, c * OCH + j0 * M * W, [[W, M], [M * W, nj], [1, W]]),
                    o_t[0:M, j0 : j0 + nj, :],
                )

            def do_tail(c, x_t, o_t, store_eng):
                u3_t = upool.tile([128, QB, CP], bf16, tag="u3")
                vb_t = psumb.tile([128, QB, CREM], f32, tag="vB")
                tiny3(x_t, NB, 1, MT, KT, vb_t)
                drain_small(vb_t, u3_t, 1, MT)
                vA = vertical3(x_t, NB, MT, KT)
                drain_big(vA, u3_t, 0, MT)
                combine(u3_t, o_t, NB, 1, MT)
                store_eng.dma_start(
                    AP(o_d, c * OCH + NB * M * W, [[W, MT], [1, W]]),
                    o_t[0:MT, NB, :],
                )

            def do_quad(c, x_t, o_t, g, store_eng, gp_last=True, vb2=None):
                u3_t = upool.tile([128, QB, CP], bf16, tag="u3")
                if vb2 is None:
                    vb_t = psumb.tile([128, QB, CREM], f32, tag="vB")
                    tiny3(x_t, QB * g, QB, M, 128, vb_t)
                    drain_small(vb_t, u3_t, QB, M)
                else:
                    drain_small(vb2, u3_t, QB, M, vq=QB * g)
                for q in range(QB):
                    vA = vertical3(x_t, QB * g + q, M, 128)
                    drain_big(vA, u3_t, q, M)
                combine(u3_t, o_t, QB * g, QB, M, gp_last=gp_last)
                store_rows(c, o_t, QB * g, QB, store_eng)

            x_tiles = [load_channel(c, split_first=(c == 0)) for c in range(C)]
            for c in range(C - 1):
                x_t = x_tiles[c]
                o_t = opool.tile([128, NB + 1, W], bf16, tag="o")
                # remainder cols of all 8 full blocks in one 3-matmul set
                vb2 = psumb.tile([128, 2 * QB, CREM], f32, tag="vB")
                tiny3(x_t, 0, 2 * QB, M, 128, vb2)
                do_quad(c, x_t, o_t, 0, nc.sync, vb2=vb2)
                do_quad(c, x_t, o_t, 1, nc.gpsimd, vb2=vb2)
                do_tail(c, x_t, o_t, nc.gpsimd)
            # last channel: tail first, then shrinking groups -> short pipe tail
            c = C - 1
            x_t = x_tiles[c]
            o_t = opool.tile([128, NB + 1, W], bf16, tag="o")
            do_tail(c, x_t, o_t, nc.gpsimd)
            do_quad(c, x_t, o_t, 0, nc.gpsimd)
            u3_t = upool.tile([128, QB, CP], bf16, tag="u3")
            vb_t = psumb.tile([128, QB, CREM], f32, tag="vB")
            tiny3(x_t, 4, 2, M, 128, vb_t)
            drain_small(vb_t, u3_t, 2, M)
            for q in range(2):
                vA = vertical3(x_t, 4 + q, M, 128)
                drain_big(vA, u3_t, q, M)
            combine(u3_t, o_t, 4, 2, M)
            store_rows(c, o_t, 4, 2, nc.sync)
            u3_t = upool.tile([128, QB, CP], bf16, tag="u3")
            vb_t = psumb.tile([128, QB, CREM], f32, tag="vB")
            tiny3(x_t, 6, 2, M, 128, vb_t)
            drain_small(vb_t, u3_t, 2, M)
            for q in range(2):
                vA = vertical3(x_t, 6 + q, M, 128)
                drain_big(vA, u3_t, q, M)
                combine(u3_t, o_t, 6 + q, 1, M, q0=q)
                store_rows(c, o_t, 6 + q, 1, nc.scalar if q else nc.sync)
    nc.compile()
    return nc


def _get_nc():
    if "nc" not in _CACHE:
        _CACHE["nc"] = _build()
    return _CACHE["nc"]


def _prepare_in_maps(tensor: np.ndarray) -> list:
    x = np.asarray(tensor, dtype=np.float32)
    assert x.shape == (B, C, H, W), x.shape
    wmat = _band_weights()
    return [{"x": _pack_image(x[i]), "w": wmat} for i in range(B)]


def kernel(tensor: np.ndarray) -> np.ndarray:
    nc = _get_nc()
    in_maps = _prepare_in_maps(tensor)
    res = run_bass_kernel_spmd(nc, in_maps, core_ids=list(range(B)))
    return np.stack(
        [res.results[i]["o"].astype(np.float32) for i in range(B)], axis=0
    )
